# revision 12
# baseline (speedup 1.0000x reference)
"""DLSA block (clustered sparse attention) Trainium2 kernel.

Full-input contract: kernel(**inputs) takes the complete unsharded tensors,
shards batch-dim across 8 NeuronCores, runs a Bass/Tile kernel per core, and
gathers the full output on host.

v3: bf16 datapath + software-pipelined pair loop.
  - All matmul operands bf16 (fp32 PSUM accumulate), rel err ~6e-3.
  - Scores for TWO groups share one 4-bank PSUM generation (8 row-banded
    matmuls; bank c holds clusters c of both groups at col offsets 0/128),
    so there is ONE exp ACTIVATE per 8 clusters and the scalar engine runs
    only exp (~33us). The exp->next-scores WAR dependency is covered by
    issuing V matmuls, the previous pair's F matmuls, and the Z matmuls of
    the next superchunk inside the window.
  - Normalization moved to host: F is stored UNNORMALIZED together with the
    softmax denominator column (33 cols per cluster); the unpack step does
    out = F/r. On-device vector work is only z-evac, V bias-add, F evac.
  - Host packs inputs into slab layout ([sc, (c4 f)=128, (j s)=1024] bf16,
    2KB DMA rows); output leaves as [s=128, (sc, pair, g, c, 33)] fp32.

Algebraic folds done on host (weight-space only, float64 for accuracy):
  A    = Wq^T @ Wk / sqrt(D)      -> scores S = Xg A Xg^T + (bq Wk/sqrt(D)) Xg^T
  bk drops entirely (softmax-invariant row constant).
  Wvo  = Wo @ Wv                  -> V' = Xp Wvo^T  (V and O projections fused)
  bo2  = bo + Wo @ bv             (bv commutes through attention; added to V')
"""

import sys

for _p in ("/opt/trn_rl_repo",):
    if _p not in sys.path:
        sys.path.insert(0, _p)

from contextlib import ExitStack

import numpy as np
import ml_dtypes

import concourse.bass as bass
import concourse.tile as tile
from concourse import bacc, mybir
from concourse.bass_utils import run_bass_kernel_spmd

F32 = mybir.dt.float32
BF16 = mybir.dt.bfloat16
NP_BF16 = ml_dtypes.bfloat16

B, N, D = 16, 16384, 32
C_TOTAL, S = 128, 128          # clusters per batch, points per cluster
N_CORES = 8
B_LOC = B // N_CORES           # batches per core
ROWS = B_LOC * N               # data rows per core
SC_CLUSTERS = 32               # clusters per superchunk
N_SC = ROWS // (SC_CLUSTERS * S)  # 8 superchunks per core
G = 4                          # clusters per group
GROUPS_PER_SC = SC_CLUSTERS // G  # 8
SLAB_COLS = GROUPS_PER_SC * S  # 1024 cols per input slab
PAIRS_PER_SC = GROUPS_PER_SC // 2  # 4
N_PAIRS = N_SC * PAIRS_PER_SC  # 32 pairs per core
FCOLS = 2 * G * 33             # 264 f-cols per pair (8 clusters x [32 vals | r])


def _build_program():
    nc = bacc.Bacc("TRN2", target_bir_lowering=False, debug=False)

    hg_p = nc.dram_tensor("hg_p", [N_SC * 128, SLAB_COLS], BF16,
                          kind="ExternalInput").ap()
    hp_p = nc.dram_tensor("hp_p", [N_SC * 128, SLAB_COLS], BF16,
                          kind="ExternalInput").ap()
    a_blk = nc.dram_tensor("a_blk", [128, 128], BF16, kind="ExternalInput").ap()
    cvec = nc.dram_tensor("cvec", [128, 1], F32, kind="ExternalInput").ap()
    wvo_blk = nc.dram_tensor("wvo_blk", [128, 128], BF16,
                             kind="ExternalInput").ap()
    bo2_rep = nc.dram_tensor("bo2_rep", [128, 2 * G * D], F32,
                             kind="ExternalInput").ap()
    out = nc.dram_tensor("out", [128, N_PAIRS * FCOLS], F32,
                         kind="ExternalOutput").ap()

    with tile.TileContext(nc) as tc, ExitStack() as ctx:
        consts = ctx.enter_context(tc.tile_pool(name="consts", bufs=1))
        io_pool = ctx.enter_context(tc.tile_pool(name="io", bufs=3))
        zsb_pool = ctx.enter_context(tc.tile_pool(name="zsb", bufs=2))
        pA_pool = ctx.enter_context(tc.tile_pool(name="pA", bufs=2))
        pB_pool = ctx.enter_context(tc.tile_pool(name="pB", bufs=2))
        out_pool = ctx.enter_context(tc.tile_pool(name="osc", bufs=2))
        v33_pool = ctx.enter_context(tc.tile_pool(name="v33", bufs=2))

        # PSUM: 8 banks of 2KB. Scores split into two 2-bank generations
        # (bands c0,c1 -> pool A; c2,c3 -> pool B) so the next pair's A-half
        # can run while expB of this pair still drains: A2 + B2 + z1 + v1 +
        # f2 = 8 banks.
        ps_z = ctx.enter_context(tc.tile_pool(name="ps_z", bufs=1, space="PSUM"))
        ps_wkA = ctx.enter_context(tc.tile_pool(name="ps_wkA", bufs=1, space="PSUM"))
        ps_wkB = ctx.enter_context(tc.tile_pool(name="ps_wkB", bufs=1, space="PSUM"))
        ps_v = ctx.enter_context(tc.tile_pool(name="ps_v", bufs=1, space="PSUM"))
        ps_f = ctx.enter_context(tc.tile_pool(name="ps_f", bufs=2, space="PSUM"))

        # constants
        a_sb = consts.tile([128, 128], BF16, tag="a_sb")
        nc.gpsimd.dma_start(a_sb[:], a_blk)
        cvec_sb = consts.tile([128, 1], F32, tag="cvec_sb")
        nc.gpsimd.dma_start(cvec_sb[:], cvec)
        wvo_sb = consts.tile([128, 128], BF16, tag="wvo_sb")
        nc.gpsimd.dma_start(wvo_sb[:], wvo_blk)
        bo2_sb = consts.tile([128, 2 * G * D], F32, tag="bo2_sb")
        nc.gpsimd.dma_start(bo2_sb[:], bo2_rep)

        # warm the exp table while the first slabs load
        warm = consts.tile([128, 1], F32, tag="warm")
        nc.scalar.activation(warm[:], cvec_sb[:],
                             mybir.ActivationFunctionType.Exp)

        # v33 ring: [t, (g c 33)] with ones in col 32 of each 33-block
        v33_tiles = []
        for i in range(2):
            t = v33_pool.tile([128, FCOLS], BF16, tag=f"v33_{i}")
            ones_ap = t[:].rearrange("p (c g) -> p c g", g=33)[:, :, 32:33]
            nc.gpsimd.memset(ones_ap, 1.0)
            v33_tiles.append(t)

        slabs = {}

        def load_slabs(sc, prologue=False):
            hg_sc = io_pool.tile([128, SLAB_COLS], BF16, tag="hg_sc")
            hp_sc = io_pool.tile([128, SLAB_COLS], BF16, tag="hp_sc")
            r0 = sc * 128
            nc.sync.dma_start(hg_sc[:], hg_p[r0 : r0 + 128, :])
            # prologue: hp rides the gpsimd queue (only tiny const loads
            # there) so hg0/hg1 are alone on sync and land in half the time
            eng = nc.gpsimd if prologue else nc.sync
            eng.dma_start(hp_sc[:], hp_p[r0 : r0 + 128, :])
            slabs[sc] = (hg_sc, hp_sc)

        zsb = {}

        def z_phase(sc, h):
            """Z'^T half-slab via one wide matmul + DVE evac (bias cvec)."""
            hg_sc, _ = slabs[sc]
            if h == 0:
                zsb[sc] = zsb_pool.tile([128, SLAB_COLS], BF16, tag="z_sb", name="z_sb")
            half = slice(h * 512, (h + 1) * 512)
            z_ps = ps_z.tile([128, 512], F32, tag="z_ps")
            nc.tensor.matmul(z_ps[:], a_sb[:], hg_sc[:, half])
            nc.vector.tensor_scalar_add(zsb[sc][:, half], z_ps[:], cvec_sb[:])

        # state carried across pairs for the lag-1 F stage
        prev = None  # (p_sb2, v33, f2 slot filled later)
        f_tiles = {}
        out_tiles = {}

        def emit_F(P):
            """F matmuls for pair P (reads p_sb2/v33 of P), into f2[P]."""
            p_sbA, p_sbB, v33 = prev_state[P]
            f2 = ps_f.tile([128, FCOLS], F32, tag="f2", name="f2")
            f_tiles[P] = f2
            for g in range(2):
                for c in range(G):
                    col = (g * G + c) * 33
                    p_sb = p_sbA if c < 2 else p_sbB
                    ci = c % 2
                    nc.tensor.matmul(
                        f2[:, col : col + 33],
                        p_sb[:, ci * 256 + g * 128 : ci * 256 + (g + 1) * 128],
                        v33[:, col : col + 33],
                        tile_position=(0, 0),
                    )

        def emit_F_evac(P):
            """Copy f2[P] (unnormalized F + r cols) into the sc out tile."""
            sc = P // PAIRS_PER_SC
            p = P % PAIRS_PER_SC
            if p == 0:
                out_tiles[sc] = out_pool.tile([128, PAIRS_PER_SC * FCOLS],
                                              F32, tag="out_sc", name="out_sc")
            nc.vector.tensor_copy(
                out_tiles[sc][:, p * FCOLS : (p + 1) * FCOLS],
                f_tiles[P][:],
            )
            del f_tiles[P]
            if p % 2 == 1:  # store each completed half-sc immediately
                h0 = (p - 1) * FCOLS
                base = sc * PAIRS_PER_SC * FCOLS
                nc.gpsimd.dma_start(
                    out[:, base + h0 : base + h0 + 2 * FCOLS],
                    out_tiles[sc][:, h0 : h0 + 2 * FCOLS],
                )

        prev_state = {}

        # prologue
        load_slabs(0, prologue=True)
        load_slabs(1, prologue=True)
        z_phase(0, 0)
        z_phase(0, 1)

        for P in range(N_PAIRS):
            sc, p = P // PAIRS_PER_SC, P % PAIRS_PER_SC
            hg_sc, hp_sc = slabs[sc]
            z_sb = zsb[sc]
            if p == 0 and sc + 1 < N_SC and sc + 1 not in slabs:
                load_slabs(sc + 1)


            # scores: 8 row-banded matmuls in two 2-bank halves.
            # Half A = bands c0,c1; half B = bands c2,c3; within a half,
            # band i (cols i*512..) holds group g at +g*128.
            p_half = []
            for half_i, (ps_pool, p_pool, c_lo) in enumerate(
                ((ps_wkA, pA_pool, 0), (ps_wkB, pB_pool, 2))
            ):
                wk = ps_pool.tile([128, 1024], F32, tag="wk", name="wk")
                for g in range(2):
                    j = p * 2 + g
                    jcols = slice(j * S, (j + 1) * S)
                    for ci in range(2):
                        c = c_lo + ci
                        p0 = c * 32
                        nc.tensor.matmul(
                            wk[:, ci * 512 + g * 128 : ci * 512 + (g + 1) * 128],
                            hg_sc[p0 : p0 + 32, jcols],
                            z_sb[p0 : p0 + 32, jcols],
                            tile_position=(p0, 0),
                        )
                wk_view = wk[:].rearrange("p (c q) -> p c q", q=512)
                p_sb = p_pool.tile([128, 512], BF16, tag="p_sb", name="p_sb")
                nc.scalar.activation(
                    p_sb[:].rearrange("p (c q) -> p c q", q=256),
                    wk_view[:, :, 0:256],
                    mybir.ActivationFunctionType.Exp,
                )
                p_half.append(p_sb)

            # V' for both groups -> v_ps [128, (g, c, 32)]
            v_ps = ps_v.tile([128, 256], F32, tag="v_ps")
            for g in range(2):
                j = p * 2 + g
                nc.tensor.matmul(
                    v_ps[:, g * 128 : (g + 1) * 128],
                    hp_sc[:, (j * S) : (j + 1) * S],
                    wvo_sb[:],
                )

            # lag-1 F matmuls fill the exp->scores window
            if P > 0:
                emit_F(P - 1)

            # Z matmuls of the next superchunk also land in the window
            if p == 2 and sc + 1 < N_SC:
                z_phase(sc + 1, 0)
                if sc + 2 < N_SC and sc + 2 not in slabs:
                    load_slabs(sc + 2)
            elif p == 3 and sc + 1 < N_SC:
                z_phase(sc + 1, 1)

            # V'' = V' + bo2 -> bf16 into the v33 ring (ones col preserved)
            v33 = v33_tiles[P % 2]
            nc.vector.tensor_tensor(
                v33[:].rearrange("p (c g) -> p c g", g=33)[:, :, 0:32],
                v_ps[:].rearrange("p (c g) -> p c g", g=D),
                bo2_sb[:].rearrange("p (c g) -> p c g", g=D),
                mybir.AluOpType.add,
            )
            prev_state[P] = (p_half[0], p_half[1], v33)

            if P > 0:
                emit_F_evac(P - 1)
                del prev_state[P - 1]

        # epilogue
        emit_F(N_PAIRS - 1)
        emit_F_evac(N_PAIRS - 1)

    nc.compile()
    return nc


_PROGRAM = None


def _get_program():
    global _PROGRAM
    if _PROGRAM is None:
        _PROGRAM = _build_program()
    return _PROGRAM


def _host_fold(Wq, bq, Wk, bk, Wv, bv, Wo, bo):
    Wq64, Wk64 = np.asarray(Wq, np.float64), np.asarray(Wk, np.float64)
    Wv64, Wo64 = np.asarray(Wv, np.float64), np.asarray(Wo, np.float64)
    bq64, bv64, bo64 = (np.asarray(x, np.float64) for x in (bq, bv, bo))
    scale = 1.0 / np.sqrt(np.float64(D))
    A = (Wq64.T @ Wk64) * scale                      # [e, f]
    c = (bq64 @ Wk64) * scale                        # [f]
    WvoT = (Wo64 @ Wv64).T                           # [e, g]
    bo2 = bo64 + Wo64 @ bv64                         # [g]
    a_blk = np.zeros((128, 128), NP_BF16)
    wvo_blk = np.zeros((128, 128), NP_BF16)
    for cc in range(G):
        a_blk[cc * D : (cc + 1) * D, cc * D : (cc + 1) * D] = A.astype(NP_BF16)
        wvo_blk[cc * D : (cc + 1) * D, cc * D : (cc + 1) * D] = WvoT.astype(
            NP_BF16
        )
    cvec = np.tile(c, G)[:, None].astype(np.float32)         # [128, 1]
    bo2_rep = np.tile(bo2, (128, 2 * G)).reshape(128, 2 * G * D).astype(
        np.float32
    )
    return a_blk, cvec, wvo_blk, bo2_rep


def _pack_slabs(x):
    """[B, N, D] -> per-batch [4*128, 1024] bf16 slab layout."""
    # [b, cblk, j, c4, s, f] -> [b, cblk, c4, f, j, s]
    y = np.asarray(x, np.float32).reshape(B, 4, GROUPS_PER_SC, G, S, D)
    y = y.transpose(0, 1, 3, 5, 2, 4).astype(NP_BF16)
    return y.reshape(B, 4 * 128, SLAB_COLS)


def make_in_maps(h_pos, h_geo, Wq, bq, Wk, bk, Wv, bv, Wo, bo):
    a_blk, cvec, wvo_blk, bo2_rep = _host_fold(Wq, bq, Wk, bk, Wv, bv, Wo, bo)
    hg_all = _pack_slabs(h_geo)
    hp_all = _pack_slabs(h_pos)
    in_maps = []
    for core in range(N_CORES):
        b0 = core * B_LOC
        in_maps.append(
            {
                "hg_p": np.ascontiguousarray(
                    hg_all[b0 : b0 + B_LOC].reshape(N_SC * 128, SLAB_COLS)
                ),
                "hp_p": np.ascontiguousarray(
                    hp_all[b0 : b0 + B_LOC].reshape(N_SC * 128, SLAB_COLS)
                ),
                "a_blk": a_blk,
                "cvec": cvec,
                "wvo_blk": wvo_blk,
                "bo2_rep": bo2_rep,
            }
        )
    return in_maps


def _unpack_out(out_dram):
    """[128, N_PAIRS*264] fp32 unnormalized -> [B_LOC, N, D] (host divide)."""
    y = out_dram.reshape(S, N_PAIRS, 2, G, 33)  # [s, pair, g, c, 33]
    f = y[..., :32]
    r = y[..., 32:33]
    o = f / r                                   # softmax normalization
    # pair,g -> group j (2 per pair); cluster = (sc, j, c)
    o = o.transpose(1, 2, 3, 0, 4)              # [pair, g, c, s, d]
    return o.reshape(B_LOC, N, D)


def kernel(h_pos, h_geo, n_clusters, Wq, bq, Wk, bk, Wv, bv, Wo, bo, **kwargs):
    assert int(n_clusters) == C_TOTAL
    nc = _get_program()
    in_maps = make_in_maps(h_pos, h_geo, Wq, bq, Wk, bk, Wv, bv, Wo, bo)
    res = run_bass_kernel_spmd(nc, in_maps, core_ids=list(range(N_CORES)))
    shards = [_unpack_out(r["out"]) for r in res.results]
    return np.concatenate(shards, axis=0).astype(np.float32)


# revision 14
# speedup vs baseline: 1.1161x; 1.1161x over previous
"""DLSA block (clustered sparse attention) Trainium2 kernel.

Full-input contract: kernel(**inputs) takes the complete unsharded tensors,
shards batch-dim across 8 NeuronCores, runs a Bass/Tile kernel per core, and
gathers the full output on host.

v3: bf16 datapath + software-pipelined pair loop.
  - All matmul operands bf16 (fp32 PSUM accumulate), rel err ~6e-3.
  - Scores for TWO groups share one 4-bank PSUM generation (8 row-banded
    matmuls; bank c holds clusters c of both groups at col offsets 0/128),
    so there is ONE exp ACTIVATE per 8 clusters and the scalar engine runs
    only exp (~33us). The exp->next-scores WAR dependency is covered by
    issuing V matmuls, the previous pair's F matmuls, and the Z matmuls of
    the next superchunk inside the window.
  - Normalization moved to host: F is stored UNNORMALIZED together with the
    softmax denominator column (33 cols per cluster); the unpack step does
    out = F/r. On-device vector work is only z-evac, V bias-add, F evac.
  - Host packs inputs into slab layout ([sc, (c4 f)=128, (j s)=1024] bf16,
    2KB DMA rows); output leaves as [s=128, (sc, pair, g, c, 33)] fp32.

Algebraic folds done on host (weight-space only, float64 for accuracy):
  A    = Wq^T @ Wk / sqrt(D)      -> scores S = Xg A Xg^T + (bq Wk/sqrt(D)) Xg^T
  bk drops entirely (softmax-invariant row constant).
  Wvo  = Wo @ Wv                  -> V' = Xp Wvo^T  (V and O projections fused)
  bo2  = bo + Wo @ bv             (bv commutes through attention; added to V')
"""

import sys

for _p in ("/opt/trn_rl_repo",):
    if _p not in sys.path:
        sys.path.insert(0, _p)

from contextlib import ExitStack

import numpy as np
import ml_dtypes

import concourse.bass as bass
import concourse.tile as tile
from concourse import bacc, mybir
from concourse.bass_utils import run_bass_kernel_spmd

F32 = mybir.dt.float32
BF16 = mybir.dt.bfloat16
NP_BF16 = ml_dtypes.bfloat16

B, N, D = 16, 16384, 32
C_TOTAL, S = 128, 128          # clusters per batch, points per cluster
N_CORES = 8
B_LOC = B // N_CORES           # batches per core
ROWS = B_LOC * N               # data rows per core
SC_CLUSTERS = 32               # clusters per superchunk
N_SC = ROWS // (SC_CLUSTERS * S)  # 8 superchunks per core
G = 4                          # clusters per group
GROUPS_PER_SC = SC_CLUSTERS // G  # 8
SLAB_COLS = GROUPS_PER_SC * S  # 1024 cols per input slab
PAIRS_PER_SC = GROUPS_PER_SC // 2  # 4
N_PAIRS = N_SC * PAIRS_PER_SC  # 32 pairs per core
FCOLS = 2 * G * 33             # 264 f-cols per pair (8 clusters x [32 vals | r])


def _build_program():
    nc = bacc.Bacc("TRN2", target_bir_lowering=False, debug=False)

    hg_p = nc.dram_tensor("hg_p", [N_SC * 128, SLAB_COLS], BF16,
                          kind="ExternalInput").ap()
    hp_p = nc.dram_tensor("hp_p", [N_SC * 128, SLAB_COLS], BF16,
                          kind="ExternalInput").ap()
    a_blk = nc.dram_tensor("a_blk", [128, 128], BF16, kind="ExternalInput").ap()
    cvec = nc.dram_tensor("cvec", [128, 1], F32, kind="ExternalInput").ap()
    wvo_blk = nc.dram_tensor("wvo_blk", [128, 128], BF16,
                             kind="ExternalInput").ap()
    bo2_rep = nc.dram_tensor("bo2_rep", [128, 2 * G * D], F32,
                             kind="ExternalInput").ap()
    out = nc.dram_tensor("out", [128, N_PAIRS * FCOLS], F32,
                         kind="ExternalOutput").ap()

    with tile.TileContext(nc) as tc, ExitStack() as ctx:
        consts = ctx.enter_context(tc.tile_pool(name="consts", bufs=1))
        io_pool = ctx.enter_context(tc.tile_pool(name="io", bufs=2))
        zsb_pool = ctx.enter_context(tc.tile_pool(name="zsb", bufs=2))
        pA_pool = ctx.enter_context(tc.tile_pool(name="pA", bufs=2))
        pB_pool = ctx.enter_context(tc.tile_pool(name="pB", bufs=2))
        out_pool = ctx.enter_context(tc.tile_pool(name="osc", bufs=2))
        v33_pool = ctx.enter_context(tc.tile_pool(name="v33", bufs=2))

        # PSUM: 8 banks of 2KB. Scores split into two 2-bank generations
        # (bands c0,c1 -> pool A; c2,c3 -> pool B) so the next pair's A-half
        # can run while expB of this pair still drains: A2 + B2 + z1 + v1 +
        # f2 = 8 banks.
        ps_z = ctx.enter_context(tc.tile_pool(name="ps_z", bufs=1, space="PSUM"))
        ps_wkA = ctx.enter_context(tc.tile_pool(name="ps_wkA", bufs=1, space="PSUM"))
        ps_wkB = ctx.enter_context(tc.tile_pool(name="ps_wkB", bufs=1, space="PSUM"))
        ps_v = ctx.enter_context(tc.tile_pool(name="ps_v", bufs=1, space="PSUM"))
        ps_f = ctx.enter_context(tc.tile_pool(name="ps_f", bufs=2, space="PSUM"))

        # constants
        a_sb = consts.tile([128, 128], BF16, tag="a_sb")
        nc.gpsimd.dma_start(a_sb[:], a_blk)
        cvec_sb = consts.tile([128, 1], F32, tag="cvec_sb")
        nc.gpsimd.dma_start(cvec_sb[:], cvec)
        wvo_sb = consts.tile([128, 128], BF16, tag="wvo_sb")
        nc.gpsimd.dma_start(wvo_sb[:], wvo_blk)
        bo2_sb = consts.tile([128, 2 * G * D], F32, tag="bo2_sb")
        nc.gpsimd.dma_start(bo2_sb[:], bo2_rep)

        # warm the exp table while the first slabs load
        warm = consts.tile([128, 1], F32, tag="warm")
        nc.scalar.activation(warm[:], cvec_sb[:],
                             mybir.ActivationFunctionType.Exp)

        # v33 ring: [t, (g c 33)] with ones in col 32 of each 33-block
        v33_tiles = []
        for i in range(2):
            t = v33_pool.tile([128, FCOLS], BF16, tag=f"v33_{i}")
            ones_ap = t[:].rearrange("p (c g) -> p c g", g=33)[:, :, 32:33]
            nc.gpsimd.memset(ones_ap, 1.0)
            v33_tiles.append(t)

        slabs = {}

        def load_slabs(sc, prologue=False):
            hg_sc = io_pool.tile([128, SLAB_COLS], BF16, tag="hg_sc")
            hp_sc = io_pool.tile([128, SLAB_COLS], BF16, tag="hp_sc")
            r0 = sc * 128
            # hg in halves: z_phase(sc, 0) only reads cols 0:512, so it
            # unblocks after half the transfer time
            nc.sync.dma_start(hg_sc[:, 0:512], hg_p[r0 : r0 + 128, 0:512])
            nc.sync.dma_start(hg_sc[:, 512:1024], hg_p[r0 : r0 + 128, 512:1024])
            # prologue: hp rides the gpsimd queue (only tiny const loads
            # there) so hg0/hg1 are alone on sync and land in half the time
            eng = nc.gpsimd if prologue else nc.sync
            eng.dma_start(hp_sc[:], hp_p[r0 : r0 + 128, :])
            slabs[sc] = (hg_sc, hp_sc)

        zsb = {}

        def z_phase(sc, h):
            """Z'^T half-slab via one wide matmul + DVE evac (bias cvec)."""
            hg_sc, _ = slabs[sc]
            if h == 0:
                zsb[sc] = zsb_pool.tile([128, SLAB_COLS], BF16, tag="z_sb", name="z_sb")
            half = slice(h * 512, (h + 1) * 512)
            z_ps = ps_z.tile([128, 512], F32, tag="z_ps")
            nc.tensor.matmul(z_ps[:], a_sb[:], hg_sc[:, half])
            nc.vector.tensor_scalar_add(zsb[sc][:, half], z_ps[:], cvec_sb[:])

        # state carried across pairs for the lag-1 F stage
        prev = None  # (p_sb2, v33, f2 slot filled later)
        f_tiles = {}
        out_tiles = {}

        def emit_F(P):
            """F matmuls for pair P (reads p_sb2/v33 of P), into f2[P]."""
            p_sbA, p_sbB, v33 = prev_state[P]
            f2 = ps_f.tile([128, FCOLS], F32, tag="f2", name="f2")
            f_tiles[P] = f2
            for g in range(2):
                for c in range(G):
                    col = (g * G + c) * 33
                    p_sb = p_sbA if c < 2 else p_sbB
                    ci = c % 2
                    nc.tensor.matmul(
                        f2[:, col : col + 33],
                        p_sb[:, ci * 256 + g * 128 : ci * 256 + (g + 1) * 128],
                        v33[:, col : col + 33],
                        tile_position=(0, 0),
                    )

        def emit_F_evac(P):
            """Copy f2[P] (unnormalized F + r cols) into the sc out tile."""
            sc = P // PAIRS_PER_SC
            p = P % PAIRS_PER_SC
            if p == 0:
                out_tiles[sc] = out_pool.tile([128, PAIRS_PER_SC * FCOLS],
                                              F32, tag="out_sc", name="out_sc")
            nc.vector.tensor_copy(
                out_tiles[sc][:, p * FCOLS : (p + 1) * FCOLS],
                f_tiles[P][:],
            )
            del f_tiles[P]
            if p % 2 == 1:  # store each completed half-sc immediately
                h0 = (p - 1) * FCOLS
                base = sc * PAIRS_PER_SC * FCOLS
                nc.gpsimd.dma_start(
                    out[:, base + h0 : base + h0 + 2 * FCOLS],
                    out_tiles[sc][:, h0 : h0 + 2 * FCOLS],
                )

        prev_state = {}

        # prologue
        load_slabs(0, prologue=True)
        load_slabs(1, prologue=True)
        z_phase(0, 0)
        z_phase(0, 1)

        for P in range(N_PAIRS):
            sc, p = P // PAIRS_PER_SC, P % PAIRS_PER_SC
            hg_sc, hp_sc = slabs[sc]
            z_sb = zsb[sc]
            if p == 0 and sc + 1 < N_SC and sc + 1 not in slabs:
                load_slabs(sc + 1)


            # scores: 8 row-banded matmuls in two 2-bank halves.
            # Half A = bands c0,c1; half B = bands c2,c3; within a half,
            # band i (cols i*512..) holds group g at +g*128.
            p_half = []
            for half_i, (ps_pool, p_pool, c_lo) in enumerate(
                ((ps_wkA, pA_pool, 0), (ps_wkB, pB_pool, 2))
            ):
                wk = ps_pool.tile([128, 1024], F32, tag="wk", name="wk")
                for g in range(2):
                    j = p * 2 + g
                    jcols = slice(j * S, (j + 1) * S)
                    for ci in range(2):
                        c = c_lo + ci
                        p0 = c * 32
                        nc.tensor.matmul(
                            wk[:, ci * 512 + g * 128 : ci * 512 + (g + 1) * 128],
                            hg_sc[p0 : p0 + 32, jcols],
                            z_sb[p0 : p0 + 32, jcols],
                            tile_position=(p0, 0),
                        )
                wk_view = wk[:].rearrange("p (c q) -> p c q", q=512)
                p_sb = p_pool.tile([128, 512], BF16, tag="p_sb", name="p_sb")
                nc.scalar.activation(
                    p_sb[:].rearrange("p (c q) -> p c q", q=256),
                    wk_view[:, :, 0:256],
                    mybir.ActivationFunctionType.Exp,
                )
                p_half.append(p_sb)

            # V' for both groups -> v_ps [128, (g, c, 32)]
            v_ps = ps_v.tile([128, 256], F32, tag="v_ps")
            for g in range(2):
                j = p * 2 + g
                nc.tensor.matmul(
                    v_ps[:, g * 128 : (g + 1) * 128],
                    hp_sc[:, (j * S) : (j + 1) * S],
                    wvo_sb[:],
                )

            # lag-1 F matmuls fill the exp->scores window
            if P > 0:
                emit_F(P - 1)

            # Z matmuls of the next superchunk also land in the window
            if p == 2 and sc + 1 < N_SC:
                z_phase(sc + 1, 0)
            elif p == 3 and sc + 1 < N_SC:
                z_phase(sc + 1, 1)

            # V'' = V' + bo2 -> bf16 into the v33 ring (ones col preserved)
            v33 = v33_tiles[P % 2]
            nc.vector.tensor_tensor(
                v33[:].rearrange("p (c g) -> p c g", g=33)[:, :, 0:32],
                v_ps[:].rearrange("p (c g) -> p c g", g=D),
                bo2_sb[:].rearrange("p (c g) -> p c g", g=D),
                mybir.AluOpType.add,
            )
            prev_state[P] = (p_half[0], p_half[1], v33)

            if P > 0:
                emit_F_evac(P - 1)
                del prev_state[P - 1]

        # epilogue
        emit_F(N_PAIRS - 1)
        emit_F_evac(N_PAIRS - 1)

    nc.compile()
    return nc


_PROGRAM = None


def _get_program():
    global _PROGRAM
    if _PROGRAM is None:
        _PROGRAM = _build_program()
    return _PROGRAM


def _host_fold(Wq, bq, Wk, bk, Wv, bv, Wo, bo):
    Wq64, Wk64 = np.asarray(Wq, np.float64), np.asarray(Wk, np.float64)
    Wv64, Wo64 = np.asarray(Wv, np.float64), np.asarray(Wo, np.float64)
    bq64, bv64, bo64 = (np.asarray(x, np.float64) for x in (bq, bv, bo))
    scale = 1.0 / np.sqrt(np.float64(D))
    A = (Wq64.T @ Wk64) * scale                      # [e, f]
    c = (bq64 @ Wk64) * scale                        # [f]
    WvoT = (Wo64 @ Wv64).T                           # [e, g]
    bo2 = bo64 + Wo64 @ bv64                         # [g]
    a_blk = np.zeros((128, 128), NP_BF16)
    wvo_blk = np.zeros((128, 128), NP_BF16)
    for cc in range(G):
        a_blk[cc * D : (cc + 1) * D, cc * D : (cc + 1) * D] = A.astype(NP_BF16)
        wvo_blk[cc * D : (cc + 1) * D, cc * D : (cc + 1) * D] = WvoT.astype(
            NP_BF16
        )
    cvec = np.tile(c, G)[:, None].astype(np.float32)         # [128, 1]
    bo2_rep = np.tile(bo2, (128, 2 * G)).reshape(128, 2 * G * D).astype(
        np.float32
    )
    return a_blk, cvec, wvo_blk, bo2_rep


def _pack_slabs(x):
    """[B, N, D] -> per-batch [4*128, 1024] bf16 slab layout."""
    # [b, cblk, j, c4, s, f] -> [b, cblk, c4, f, j, s]
    y = np.asarray(x, np.float32).reshape(B, 4, GROUPS_PER_SC, G, S, D)
    y = y.transpose(0, 1, 3, 5, 2, 4).astype(NP_BF16)
    return y.reshape(B, 4 * 128, SLAB_COLS)


def make_in_maps(h_pos, h_geo, Wq, bq, Wk, bk, Wv, bv, Wo, bo):
    a_blk, cvec, wvo_blk, bo2_rep = _host_fold(Wq, bq, Wk, bk, Wv, bv, Wo, bo)
    hg_all = _pack_slabs(h_geo)
    hp_all = _pack_slabs(h_pos)
    in_maps = []
    for core in range(N_CORES):
        b0 = core * B_LOC
        in_maps.append(
            {
                "hg_p": np.ascontiguousarray(
                    hg_all[b0 : b0 + B_LOC].reshape(N_SC * 128, SLAB_COLS)
                ),
                "hp_p": np.ascontiguousarray(
                    hp_all[b0 : b0 + B_LOC].reshape(N_SC * 128, SLAB_COLS)
                ),
                "a_blk": a_blk,
                "cvec": cvec,
                "wvo_blk": wvo_blk,
                "bo2_rep": bo2_rep,
            }
        )
    return in_maps


def _unpack_out(out_dram):
    """[128, N_PAIRS*264] fp32 unnormalized -> [B_LOC, N, D] (host divide)."""
    y = out_dram.reshape(S, N_PAIRS, 2, G, 33)  # [s, pair, g, c, 33]
    f = y[..., :32]
    r = y[..., 32:33]
    o = f / r                                   # softmax normalization
    # pair,g -> group j (2 per pair); cluster = (sc, j, c)
    o = o.transpose(1, 2, 3, 0, 4)              # [pair, g, c, s, d]
    return o.reshape(B_LOC, N, D)


def kernel(h_pos, h_geo, n_clusters, Wq, bq, Wk, bk, Wv, bv, Wo, bo, **kwargs):
    assert int(n_clusters) == C_TOTAL
    nc = _get_program()
    in_maps = make_in_maps(h_pos, h_geo, Wq, bq, Wk, bk, Wv, bv, Wo, bo)
    res = run_bass_kernel_spmd(nc, in_maps, core_ids=list(range(N_CORES)))
    shards = [_unpack_out(r["out"]) for r in res.results]
    return np.concatenate(shards, axis=0).astype(np.float32)
